# revision 1
# baseline (speedup 1.0000x reference)
"""LIF spiking-neuron (BaseNeuron) forward kernel for Trainium2.

Reference semantics (per element, over T=16 timesteps):
    decay_s = sigmoid(decay)                     # scalar
    mem_t   = mem_{t-1} * decay_s * (1 - spike_{t-1}) + x_t
    spike_t = (mem_t > 0.5)
    out     = spikes (0.0/1.0 fp32), clipped to [0,1] (no-op)

Sharding: pure data parallel over batch B=16 across 8 NeuronCores
(2 batch rows per core). Per core the shard [2, 64, 16, 64, 64] is
viewed as [128 rows=(b,c), 16*4096 cols=(t,h,w)]; for each t the
[128, 4096] slice is one SBUF tile and the recurrence runs across the
16 tiles.

Per-timestep compute (DVE only, with R = "retained" mem*(mem<=0.5)):
    M_t = (R_{t-1} * decay_s) + X_t     scalar_tensor_tensor(mult, add)
    S_t = (M_t > 0.5)                   tensor_scalar(is_gt)  -> output
    R_t = (M_t <= 0.5) * M_t            scalar_tensor_tensor(is_le, mult)

This is bit-identical to the reference's fp32 rounding sequence
(mask mult by exact 0.0/1.0 commutes exactly with the decay mult).
"""

import os
import sys

sys.path.insert(0, "/opt/trn_rl_repo")

import numpy as np

_N_CORES = 8
_B, _C, _T, _H, _W = 16, 64, 16, 64, 64
_BPC = _B // _N_CORES            # batch rows per core = 2
_ROWS = _BPC * _C                # 128 partitions
_FD = _H * _W                    # 4096 elements per t per partition
_THRESH = 0.5
_CH = 2                          # timesteps per DMA chunk

# filled by kernel() when tracing is enabled via BASS_KERNEL_TRACE=1
last_results = None

_cache = {}


def _build(decay_s: float, repeat: int = 1, spike_engine: str = "act"):
    import concourse.bass as bass
    import concourse.tile as tile
    from concourse import mybir
    from contextlib import ExitStack

    f32 = mybir.dt.float32
    Alu = mybir.AluOpType
    Act = mybir.ActivationFunctionType

    nc = bass.Bass("TRN2", target_bir_lowering=False, debug=False)
    x_d = nc.dram_tensor("x", [_ROWS, _T * _FD], f32, kind="ExternalInput").ap()
    o_d = nc.dram_tensor("out", [_ROWS, _T * _FD], f32, kind="ExternalOutput").ap()

    with tile.TileContext(nc) as tc, ExitStack() as ctx:
        xp = ctx.enter_context(tc.tile_pool(name="xin", bufs=2))
        sp = ctx.enter_context(tc.tile_pool(name="spk", bufs=2))
        mp = ctx.enter_context(tc.tile_pool(name="mem", bufs=2))
        rp = ctx.enter_context(
            tc.tile_pool(name="ret", bufs=1 if spike_engine == "act" else 2)
        )
        if spike_engine == "act":
            cp = ctx.enter_context(tc.tile_pool(name="cst", bufs=1))
            nthr = cp.tile([_ROWS, 1], f32)
            nc.vector.memset(nthr[:], -_THRESH)

        for _rep in range(repeat):
            R_prev = None
            Xc = Sc = None
            for t in range(_T):
                ci = t % _CH
                last_chunk = t // _CH == _T // _CH - 1
                if ci == 0:
                    Xc = xp.tile([_ROWS, _CH * _FD], f32)
                    if _rep == 0 and t == 0:
                        # split the very first chunk so compute starts after
                        # the first timestep lands instead of the whole chunk
                        for h in range(_CH):
                            nc.scalar.dma_start(
                                Xc[:, bass.ts(h, _FD)], x_d[:, bass.ts(t + h, _FD)]
                            )
                    else:
                        nc.scalar.dma_start(
                            Xc[:], x_d[:, bass.ts(t // _CH, _CH * _FD)]
                        )
                    Sc = sp.tile([_ROWS, _CH * _FD], f32)
                xt = Xc[:, bass.ts(ci, _FD)]
                st = Sc[:, bass.ts(ci, _FD)]

                if t == 0:
                    M = xt
                else:
                    Mt = mp.tile([_ROWS, _FD], f32)
                    nc.vector.scalar_tensor_tensor(
                        Mt[:], R_prev, decay_s, xt, Alu.mult, Alu.add
                    )
                    M = Mt[:]

                # spike output: S = (M > 0.5), exactly 0.0/1.0.  Relu writes
                # straight into the output slice, Sign runs in place.  The
                # last timestep's spike runs on the (by then idle) DVE
                # instead, shortening the ACT tail.
                if spike_engine == "act" and t < _T - 1:
                    nc.scalar.activation(st, M, Act.Relu, bias=nthr[:], scale=1.0)
                    nc.scalar.activation(st, st, Act.Sign)
                else:
                    nc.vector.tensor_scalar(st, M, _THRESH, None, Alu.is_gt)

                if t < _T - 1:
                    Rt = rp.tile([_ROWS, _FD], f32)
                    nc.vector.scalar_tensor_tensor(
                        Rt[:], M, _THRESH, M, Alu.is_le, Alu.mult
                    )
                    R_prev = Rt[:]

                # stream spikes out on the other HWDGE queue: chunked for the
                # steady state, per-timestep for the final chunk's tail
                if last_chunk:
                    nc.sync.dma_start(o_d[:, bass.ts(t, _FD)], st)
                elif ci == _CH - 1:
                    nc.sync.dma_start(o_d[:, bass.ts(t // _CH, _CH * _FD)], Sc[:])

    _prune_redundant_waits(nc)
    return nc


def _prune_redundant_waits(nc) -> int:
    """Drop semaphore waits that are transitively implied by the instruction's
    other waits / proc program order.

    Tile's wait assignment is per-proc minimal but not transitively minimal
    (documented), and this walrus build rejects DMACopy instructions carrying
    more than one sync-wait command.  Reasoning model: every instruction
    belongs to a serial proc (engine, or DMA issue queue).  A wait (s >= v)
    observed by instruction i guarantees completion of every update event e of
    s for which max-possible-sum-excluding-e < v, where the feasible completed
    sets are per-proc prefixes of s's updaters, and events issued on i's own
    proc at/after i are excluded.  Guarantees propagate through event
    completion closures.
    """
    from concourse import mybir

    insts = []
    inst_loc = []  # (block, local index) per instruction
    for blk in nc.m.functions[0].blocks:
        for li, ins in enumerate(blk.instructions):
            insts.append(ins)
            inst_loc.append((blk, li))

    def proc_of(ins):
        q = getattr(ins, "queue", None)
        if q:
            return ("q", q)
        return ("e", str(ins.engine))

    def waits_of(ins):
        si = ins.sync_info
        if si is None:
            return []
        return list(si.on_wait or [])

    def updates_of(ins):
        si = ins.sync_info
        if si is None:
            return []
        return list(si.on_update or [])

    def semkey(ref):
        return (str(ref.sync_type), ref.id)

    def add_value(u):
        """positive increment if u is a plain additive update, else None"""
        if u.update_mode in ("sem-add-imm", "sem-inc") and (
            u.update_value is not None and u.update_value > 0
        ):
            return u.update_value
        return None

    # pass 1: find the first non-additive update per sem ("dirty point")
    dirty_from = {}
    for idx, ins in enumerate(insts):
        for u in updates_of(ins):
            if add_value(u) is None:
                dirty_from.setdefault(semkey(u), idx)

    # forward pass
    def merge(dst, src):
        for k, v in src.items():
            if dst.get(k, -1) < v:
                dst[k] = v

    proc_g = {}          # proc -> guarantee dict {semkey: value}
    events = {}          # semkey -> list of (idx, proc, inc, cum_after, guarantees)
    n_pruned = 0
    splits = []          # (flat idx, instruction, waits to move out)

    for idx, ins in enumerate(insts):
        p = proc_of(ins)
        base = dict(proc_g.get(p, {}))

        def resolve(w):
            """guarantees implied by wait w at instruction idx on proc p"""
            k = semkey(w)
            out = {}
            if w.wait_mode != "sem-ge-imm" or w.wait_value is None:
                return out
            v = w.wait_value
            out[k] = v
            if k in dirty_from and dirty_from[k] <= idx:
                return out
            evs = [e for e in events.get(k, []) if not (e[1] == p and e[0] >= idx)]
            total = sum(e[2] for e in evs)
            proc_total = {}
            for e in evs:
                proc_total[e[1]] = proc_total.get(e[1], 0) + e[2]
            # event e is guaranteed-complete iff even with every other proc
            # fully done and e's own proc stopped just before e, v can't be
            # reached: (total - proc_total[e.proc] + prefix_before_e) < v
            prefix = {}
            for e in evs:
                pre = prefix.get(e[1], 0)
                if total - proc_total[e[1]] + pre < v:
                    merge(out, e[4])
                prefix[e[1]] = pre + e[2]
            return out

        ws = waits_of(ins)
        if len(ws) > 1:
            # try to prune redundant waits
            keep = list(ws)
            changed = True
            while changed and len(keep) > 1:
                changed = False
                for j, w in enumerate(keep):
                    if w.wait_mode != "sem-ge-imm" or w.wait_value is None:
                        continue
                    g = dict(base)
                    for k2, w2 in enumerate(keep):
                        if k2 != j:
                            merge(g, resolve(w2))
                    if g.get(semkey(w), -1) >= w.wait_value:
                        keep.pop(j)
                        n_pruned += 1
                        changed = True
                        break
            if len(keep) != len(ws):
                ins.sync_info.on_wait = keep
                ws = keep
            if len(keep) > 1:
                # this walrus build accepts at most one sync-wait command per
                # instruction: move the extras onto standalone EventSemaphore
                # instructions placed just before it on the same engine
                splits.append((idx, ins, keep[:-1]))
                ins.sync_info.on_wait = keep[-1:]

        # start guarantees (use the original semantics: all kept waits hold)
        g_start = dict(base)
        for w in ws:
            merge(g_start, resolve(w))

        for u in updates_of(ins):
            k = semkey(u)
            if k in dirty_from and dirty_from[k] <= idx:
                continue
            inc = add_value(u)
            if inc is not None:
                evs = events.setdefault(k, [])
                cum = (evs[-1][3] if evs else 0) + inc
                ev_g = dict(g_start)
                ev_g[k] = cum
                evs.append((idx, p, inc, cum, ev_g))

        # Successors on this proc inherit only the guarantees observed at
        # dispatch (g_start).  An instruction's own sem updates fire at
        # write-ack, which is asynchronous wrt the next instruction on the
        # same engine — that's why Tile emits same-engine waits, and we must
        # not treat them as implied by program order.
        proc_g[p] = g_start

    # insert EventSemaphore carriers for the moved waits (per block, back to
    # front so local indices stay valid)
    by_block = {}
    for idx, ins, moved in splits:
        blk, li = inst_loc[idx]
        by_block.setdefault(id(blk), (blk, []))[1].append((li, ins, moved))
    for blk, items in by_block.values():
        new_insts = list(blk.instructions)
        for li, ins, moved in sorted(items, key=lambda x: -x[0]):
            carriers = [
                mybir.InstEventSemaphore(
                    name=nc.get_next_instruction_name(),
                    engine=ins.engine,
                    sync_info=mybir.SyncInfo(on_wait=[w], on_update=[]),
                )
                for w in moved
            ]
            for c in carriers:
                nc.inst_map[c.name] = c
            new_insts[li:li] = carriers
        blk.instructions = new_insts

    return n_pruned


def _sigmoid_f32(v: np.ndarray) -> float:
    # fp32 sigmoid; bit-identical to jax CPU jax.nn.sigmoid for this input
    # (the on-device ACT-table sigmoid is ~36 ULP off — don't use it)
    v32 = np.float32(np.asarray(v).reshape(-1)[0])
    return float(np.float32(1.0) / (np.float32(1.0) + np.exp(-v32, dtype=np.float32)))


def kernel(x: np.ndarray, decay: np.ndarray) -> np.ndarray:
    global last_results
    from concourse.bass_utils import run_bass_kernel_spmd

    x = np.ascontiguousarray(np.asarray(x, dtype=np.float32))
    assert x.shape == (_B, _C, _T, _H, _W), x.shape
    decay_s = _sigmoid_f32(np.asarray(decay, dtype=np.float32))

    nc = _cache.get(decay_s)
    if nc is None:
        nc = _cache[decay_s] = _build(decay_s)

    shards = [
        x[i * _BPC : (i + 1) * _BPC].reshape(_ROWS, _T * _FD)
        for i in range(_N_CORES)
    ]
    in_maps = [{"x": s} for s in shards]

    res = run_bass_kernel_spmd(nc, in_maps, list(range(_N_CORES)), trace=False)
    last_results = res

    out = np.concatenate(
        [r["out"].reshape(_BPC, _C, _T, _H, _W) for r in res.results], axis=0
    )
    return np.ascontiguousarray(out.astype(np.float32))

